# revision 1
# baseline (speedup 1.0000x reference)
# Gaussian-kernel ridge-regression matvec on 8 Trainium2 cores.
#
#   out_i = sum_j exp(-||x_i - y_j||^2 / g) * alpha_j
#   N=8192 queries, M=16384 train points, DIM=32, g scalar.
#
# Factorization (host prep is O(N+M), device does the O(N*M) part):
#   exp(-(x^2+y^2-2xy)/g)*a_j = exp(-x_i^2/g) * sign(a_j) * exp(s_ij),
#   s_ij = (2/g) x_i.y_j + c_j,   c_j = -y_j^2/g + ln|a_j|
# Train points are host-sorted so sign(a)>0 comes first (npos); the device
# computes s via an augmented K=33 matmul (row 32 of x~ is 1, row 32 of y~
# is c_j), exps it, and row-sums the pos and neg column ranges separately.
# Row scale exp(-x_i^2/g) is applied on host.
#
# The matmul runs in fp16 hi/lo "triple" form for near-fp32 accuracy at
# 1 cycle/row:  x.y ~= xh.yh + xh.yl + xl.yh  (xl*yl ~ 2^-22, dropped),
# accumulated in PSUM fp32.
#
# Per core (1024 rows): 8 i-tiles x 8 groups of 2048 cols; each group =
# 4 PSUM banks x 3 accumulating matmuls; ACT exp in-place on PSUM with
# accum_out giving per-row sums per segment; tiny DVE reduce/sub; one DMA.

import numpy as np

N, M, DIM, NCORES = 8192, 16384, 32, 8
NLOC = N // NCORES
ITILES = NLOC // 128
GRP = 2048
NGRP = M // GRP
KAUG = DIM + 1

_cache = {}


def _build(npos):
    import concourse.bass as bass
    import concourse.tile as tile
    from concourse import bacc, mybir

    f32 = mybir.dt.float32
    f16 = mybir.dt.float16
    Exp = mybir.ActivationFunctionType.Exp
    X = mybir.AxisListType.X

    nc = bacc.Bacc("TRN2", target_bir_lowering=False, debug=False)
    yh = nc.dram_tensor("yh", [KAUG, M], f16, kind="ExternalInput").ap()
    yl = nc.dram_tensor("yl", [KAUG, M], f16, kind="ExternalInput").ap()
    xh = nc.dram_tensor("xh", [KAUG, NLOC], f16, kind="ExternalInput").ap()
    xl = nc.dram_tensor("xl", [KAUG, NLOC], f16, kind="ExternalInput").ap()
    o = nc.dram_tensor("o", [128, ITILES], f32, kind="ExternalOutput").ap()

    segs = []
    for gi in range(NGRP):
        g0, g1 = gi * GRP, (gi + 1) * GRP
        if g0 < npos:
            segs.append((g0, min(g1, npos), True))
        if g1 > npos:
            segs.append((max(g0, npos), g1, False))
    npos_segs = sum(1 for s in segs if s[2])
    nseg = len(segs)

    with tile.TileContext(nc) as tc:
        with tc.tile_pool(name="ypool", bufs=1) as ypool, \
             tc.tile_pool(name="xpool", bufs=1) as xpool, \
             tc.tile_pool(name="psum", bufs=2, space="PSUM") as pp, \
             tc.tile_pool(name="parts", bufs=ITILES) as partp, \
             tc.tile_pool(name="small", bufs=2 * ITILES) as smallp, \
             tc.tile_pool(name="res", bufs=1) as resp:

            YCH = 4096
            yhts, ylts = [], []
            for ci in range(M // YCH):
                t = ypool.tile([KAUG, YCH], f16, tag=f"yh{ci}")
                nc.sync.dma_start(t[:], yh[:, bass.ts(ci, YCH)])
                yhts.append(t)
                t = ypool.tile([KAUG, YCH], f16, tag=f"yl{ci}")
                nc.sync.dma_start(t[:], yl[:, bass.ts(ci, YCH)])
                ylts.append(t)
            xht = xpool.tile([KAUG, NLOC], f16, tag="xh")
            nc.sync.dma_start(xht[:], xh[:])
            xlt = xpool.tile([KAUG, NLOC], f16, tag="xl")
            nc.sync.dma_start(xlt[:], xl[:])

            # Pre-touch all DMA'd tiles on the PE so real matmuls never carry
            # DMA-queue waits (walrus limits sync waits per matmul to 2, and
            # slot-recycling matmuls already need ACT+PE waits).
            dummyw = smallp.tile([KAUG, 1], f16, tag="dummyw")
            nc.vector.memset(dummyw[:], 0.0)
            dpsum = pp.tile([1, 512], f32, tag="ps")
            nc.tensor.matmul(dpsum[:, 0:1], dummyw[:], dummyw[:],
                             start=True, stop=True)
            for di, t in enumerate(yhts + ylts + [xht, xlt]):
                nc.tensor.matmul(dpsum[:, di + 1:di + 2], dummyw[:], t[:, 0:1],
                                 start=True, stop=True)

            res = resp.tile([128, ITILES], f32)

            for it in range(ITILES):
                xhw = xht[:, bass.ts(it, 128)]
                xlw = xlt[:, bass.ts(it, 128)]
                parts = partp.tile([128, nseg], f32, tag="parts")

                seg_i = 0
                for gi in range(NGRP):
                    ps = pp.tile([128, GRP], f32, tag="ps")
                    # strided memset: one element in each of the 4 banks ->
                    # this DVE op becomes the tile's first accessor and absorbs
                    # the slot-release waits (matmuls may carry only 1 wait)
                    nc.vector.memset(
                        ps[:].rearrange("p (b c) -> p b c", c=512)[:, :, 0:1], 0.0)
                    g0 = gi * GRP
                    for k in range(4):
                        j0 = g0 + k * 512
                        ci, off = j0 // YCH, j0 % YCH
                        sl = ps[:, bass.ts(k, 512)]
                        yhr = yhts[ci][:, off:off + 512]
                        ylr = ylts[ci][:, off:off + 512]
                        nc.tensor.matmul(sl, xhw, yhr, start=True, stop=False)
                        nc.tensor.matmul(sl, xhw, ylr, start=False, stop=False)
                        nc.tensor.matmul(sl, xlw, yhr, start=False, stop=True)
                    while seg_i < nseg and segs[seg_i][0] < g0 + GRP:
                        s0, s1, _pos = segs[seg_i]
                        seg = ps[:, s0 - g0: s1 - g0]
                        nc.scalar.activation(seg, seg, Exp,
                                             accum_out=parts[:, seg_i:seg_i + 1])
                        seg_i += 1

                possum = smallp.tile([128, 1], f32, tag="pos")
                negsum = smallp.tile([128, 1], f32, tag="neg")
                if npos_segs:
                    nc.vector.reduce_sum(possum[:], parts[:, 0:npos_segs], axis=X)
                else:
                    nc.vector.memset(possum[:], 0.0)
                if nseg - npos_segs:
                    nc.vector.reduce_sum(negsum[:], parts[:, npos_segs:nseg], axis=X)
                else:
                    nc.vector.memset(negsum[:], 0.0)
                nc.vector.tensor_sub(res[:, it:it + 1], possum[:], negsum[:])

            nc.sync.dma_start(o[:], res[:])

    nc.compile()
    return nc


def kernel(x, y_train, alphas, g):
    from concourse.bass_utils import run_bass_kernel_spmd

    x = np.asarray(x, dtype=np.float32)
    y_train = np.asarray(y_train, dtype=np.float32)
    a = np.asarray(alphas, dtype=np.float32).reshape(-1)
    gf = float(np.asarray(g).reshape(-1)[0])

    y2 = np.sum(y_train.astype(np.float64) ** 2, axis=1)
    with np.errstate(divide="ignore"):
        c = -y2 / gf + np.log(np.abs(a.astype(np.float64)))
    c = np.maximum(c, -1e4)

    pos = a >= 0
    order = np.concatenate([np.nonzero(pos)[0], np.nonzero(~pos)[0]])
    npos = int(pos.sum())

    ytab = np.empty((KAUG, M), dtype=np.float64)
    ytab[:DIM] = (2.0 / gf) * y_train[order].T.astype(np.float64)
    ytab[DIM] = c[order]
    yh64 = ytab.astype(np.float16).astype(np.float64)
    yhn = yh64.astype(np.float16)
    yln = (ytab - yh64).astype(np.float16)

    key = npos
    if key not in _cache:
        _cache[key] = _build(npos)
    nc = _cache[key]

    in_maps = []
    for k in range(NCORES):
        xs = x[k * NLOC:(k + 1) * NLOC]
        xtab = np.empty((KAUG, NLOC), dtype=np.float64)
        xtab[:DIM] = xs.T.astype(np.float64)
        xtab[DIM] = 1.0
        xh64 = xtab.astype(np.float16).astype(np.float64)
        in_maps.append({
            "yh": yhn, "yl": yln,
            "xh": xh64.astype(np.float16),
            "xl": (xtab - xh64).astype(np.float16),
        })

    r = run_bass_kernel_spmd(nc, in_maps, core_ids=list(range(NCORES)))

    x2 = np.sum(x.astype(np.float64) ** 2, axis=1)
    rowscale = np.exp(-x2 / gf)
    out = np.empty(N, dtype=np.float64)
    for k in range(NCORES):
        out[k * NLOC:(k + 1) * NLOC] = r.results[k]["o"].T.reshape(NLOC).astype(np.float64)
    out *= rowscale
    return out.astype(np.float32).reshape(N, 1)



# revision 3
# speedup vs baseline: 1.8135x; 1.8135x over previous
# Gaussian-kernel ridge-regression matvec on 8 Trainium2 cores.
#
#   out_i = sum_j exp(-||x_i - y_j||^2 / g) * alpha_j
#   N=8192 queries, M=16384 train points, DIM=32, g scalar.
#
# Factorization (host prep is O(N+M), device does the O(N*M) part):
#   exp(-(x^2+y^2-2xy)/g)*a_j = exp(-x_i^2/g) * sign(a_j) * exp(s_ij),
#   s_ij = (2/g) x_i.y_j + c_j,   c_j = -y_j^2/g + ln|a_j|
# Train points are host-sorted so sign(a)>0 comes first (npos); the device
# computes s via an augmented K=34 fp16 matmul: rows 0-31 are the (2/g)-scaled
# y dims (x rows are the fp16 query dims), rows 32/33 carry c_j split hi/lo
# in fp16 (x rows 32/33 are 1.0) so c reaches the exp at ~fp29 accuracy while
# the dot itself is single-pass fp16 (~4e-3 abs err in s, well inside the
# 2e-2 gate). The exp runs on ACT in-place on PSUM with accum_out giving
# per-row sums per pos/neg segment; tiny DVE reduce/sub; one DMA.
# Row scale exp(-x_i^2/g) is applied on host.
#
# Per core (1024 rows): 8 i-tiles x 8 groups of 2048 cols; each group =
# one 4-bank PSUM tile filled by 2 N=1024 matmuls. ACT is the bottleneck
# (1 elem/cycle/lane); PE runs a single fp16 pass at ~1/3 the old cost.

import numpy as np

N, M, DIM, NCORES = 8192, 16384, 32, 8
NLOC = N // NCORES
ITILES = NLOC // 128
GRP = 2048
NGRP = M // GRP
KAUG = DIM + 2
MMN = 512  # matmul moving free size (walrus ISA check rejects 1024)

_cache = {}


def _build(npos):
    import concourse.bass as bass
    import concourse.tile as tile
    from concourse import bacc, mybir

    f32 = mybir.dt.float32
    f16 = mybir.dt.float16
    Exp = mybir.ActivationFunctionType.Exp
    X = mybir.AxisListType.X

    nc = bacc.Bacc("TRN2", target_bir_lowering=False, debug=False)
    yt = nc.dram_tensor("yt", [KAUG, M], f16, kind="ExternalInput").ap()
    xt = nc.dram_tensor("xt", [KAUG, NLOC], f16, kind="ExternalInput").ap()
    o = nc.dram_tensor("o", [128, ITILES], f32, kind="ExternalOutput").ap()

    segs = []
    for gi in range(NGRP):
        g0, g1 = gi * GRP, (gi + 1) * GRP
        if g0 < npos:
            segs.append((g0, min(g1, npos), True))
        if g1 > npos:
            segs.append((max(g0, npos), g1, False))
    npos_segs = sum(1 for s in segs if s[2])
    nseg = len(segs)

    with tile.TileContext(nc) as tc:
        with tc.tile_pool(name="ypool", bufs=1) as ypool, \
             tc.tile_pool(name="xpool", bufs=1) as xpool, \
             tc.tile_pool(name="psum", bufs=2, space="PSUM") as pp, \
             tc.tile_pool(name="parts", bufs=ITILES) as partp, \
             tc.tile_pool(name="small", bufs=2 * ITILES) as smallp, \
             tc.tile_pool(name="res", bufs=1) as resp:

            YCH = 4096
            yts = []
            for ci in range(M // YCH):
                t = ypool.tile([KAUG, YCH], f16, tag=f"yt{ci}")
                nc.sync.dma_start(t[:], yt[:, bass.ts(ci, YCH)])
                yts.append(t)
            xts = xpool.tile([KAUG, NLOC], f16, tag="xt")
            nc.sync.dma_start(xts[:], xt[:])

            # Pre-touch all DMA'd tiles on the PE so real matmuls never carry
            # DMA-queue waits (walrus limits sync waits per matmul to 2, and
            # slot-recycling matmuls already need ACT+PE waits).
            dummyw = smallp.tile([KAUG, 1], f16, tag="dummyw")
            nc.vector.memset(dummyw[:], 0.0)
            dpsum = pp.tile([1, 512], f32, tag="ps")
            nc.tensor.matmul(dpsum[:, 0:1], dummyw[:], dummyw[:],
                             start=True, stop=True)
            for di, t in enumerate(yts + [xts]):
                nc.tensor.matmul(dpsum[:, di + 1:di + 2], dummyw[:], t[:, 0:1],
                                 start=True, stop=True)

            res = resp.tile([128, ITILES], f32)

            for it in range(ITILES):
                xw = xts[:, bass.ts(it, 128)]
                parts = partp.tile([128, nseg], f32, tag="parts")

                seg_i = 0
                for gi in range(NGRP):
                    ps = pp.tile([128, GRP], f32, tag="ps")
                    # strided memset: one element in each of the 4 banks ->
                    # this DVE op becomes the tile's first accessor and absorbs
                    # the slot-release waits (matmuls may carry only 1 wait)
                    nc.vector.memset(
                        ps[:].rearrange("p (b c) -> p b c", c=512)[:, :, 0:1], 0.0)
                    g0 = gi * GRP
                    for h in range(GRP // MMN):
                        j0 = g0 + h * MMN
                        ci, off = j0 // YCH, j0 % YCH
                        nc.tensor.matmul(ps[:, bass.ts(h, MMN)], xw,
                                         yts[ci][:, off:off + MMN],
                                         start=True, stop=True)
                    while seg_i < nseg and segs[seg_i][0] < g0 + GRP:
                        s0, s1, _pos = segs[seg_i]
                        seg = ps[:, s0 - g0: s1 - g0]
                        nc.scalar.activation(seg, seg, Exp,
                                             accum_out=parts[:, seg_i:seg_i + 1])
                        seg_i += 1

                possum = smallp.tile([128, 1], f32, tag="pos")
                negsum = smallp.tile([128, 1], f32, tag="neg")
                if npos_segs:
                    nc.vector.reduce_sum(possum[:], parts[:, 0:npos_segs], axis=X)
                else:
                    nc.vector.memset(possum[:], 0.0)
                if nseg - npos_segs:
                    nc.vector.reduce_sum(negsum[:], parts[:, npos_segs:nseg], axis=X)
                else:
                    nc.vector.memset(negsum[:], 0.0)
                nc.vector.tensor_sub(res[:, it:it + 1], possum[:], negsum[:])

            nc.sync.dma_start(o[:], res[:])

    nc.compile()
    return nc


def kernel(x, y_train, alphas, g):
    from concourse.bass_utils import run_bass_kernel_spmd

    x = np.asarray(x, dtype=np.float32)
    y_train = np.asarray(y_train, dtype=np.float32)
    a = np.asarray(alphas, dtype=np.float32).reshape(-1)
    gf = float(np.asarray(g).reshape(-1)[0])

    y2 = np.sum(y_train.astype(np.float64) ** 2, axis=1)
    with np.errstate(divide="ignore"):
        c = -y2 / gf + np.log(np.abs(a.astype(np.float64)))
    c = np.maximum(c, -1e4)

    pos = a >= 0
    order = np.concatenate([np.nonzero(pos)[0], np.nonzero(~pos)[0]])
    npos = int(pos.sum())

    ytab = np.empty((KAUG, M), dtype=np.float64)
    ytab[:DIM] = (2.0 / gf) * y_train[order].T.astype(np.float64)
    co = c[order]
    ch = co.astype(np.float16).astype(np.float64)
    ytab[DIM] = ch
    ytab[DIM + 1] = co - ch
    ytn = ytab.astype(np.float16)

    key = npos
    if key not in _cache:
        _cache[key] = _build(npos)
    nc = _cache[key]

    in_maps = []
    for k in range(NCORES):
        xs = x[k * NLOC:(k + 1) * NLOC]
        xtab = np.empty((KAUG, NLOC), dtype=np.float64)
        xtab[:DIM] = xs.T.astype(np.float64)
        xtab[DIM] = 1.0
        xtab[DIM + 1] = 1.0
        in_maps.append({
            "yt": ytn,
            "xt": xtab.astype(np.float16),
        })

    r = run_bass_kernel_spmd(nc, in_maps, core_ids=list(range(NCORES)))

    x2 = np.sum(x.astype(np.float64) ** 2, axis=1)
    rowscale = np.exp(-x2 / gf)
    out = np.empty(N, dtype=np.float64)
    for k in range(NCORES):
        out[k * NLOC:(k + 1) * NLOC] = r.results[k]["o"].T.reshape(NLOC).astype(np.float64)
    out *= rowscale
    return out.astype(np.float32).reshape(N, 1)


# revision 5
# speedup vs baseline: 1.9216x; 1.0596x over previous
# Gaussian-kernel ridge-regression matvec on 8 Trainium2 cores.
#
#   out_i = sum_j exp(-||x_i - y_j||^2 / g) * alpha_j
#   N=8192 queries, M=16384 train points, DIM=32, g scalar.
#
# Factorization (host prep is O(N+M), device does the O(N*M) part):
#   exp(-(x^2+y^2-2xy)/g)*a_j = exp(-x_i^2/g) * sign(a_j) * exp(s_ij),
#   s_ij = (2/g) x_i.y_j + c_j,   c_j = -y_j^2/g + ln|a_j|
# Train points are host-sorted so sign(a)>0 comes first (npos); the device
# computes s via an augmented K=34 fp16 matmul: rows 0-31 are the (2/g)-scaled
# y dims (x rows are the fp16 query dims), rows 32/33 carry c_j split hi/lo
# in fp16 (x rows 32/33 are 1.0) so c reaches the exp at ~fp29 accuracy while
# the dot itself is single-pass fp16 (~4e-3 abs err in s, well inside the
# 2e-2 gate). The exp runs on ACT in-place on PSUM with accum_out giving
# per-row sums per pos/neg segment; tiny DVE reduce/sub; one DMA.
# Row scale exp(-x_i^2/g) is applied on host.
#
# Per core (1024 rows): 8 i-tiles x 8 groups of 2048 cols; each group =
# one 4-bank PSUM tile filled by 2 N=1024 matmuls. ACT is the bottleneck
# (1 elem/cycle/lane); PE runs a single fp16 pass at ~1/3 the old cost.

import numpy as np

N, M, DIM, NCORES = 8192, 16384, 32, 8
NLOC = N // NCORES
ITILES = NLOC // 128
GRP = 2048
NGRP = M // GRP
KAUG = DIM + 2
MMN = 512  # matmul moving free size (walrus ISA check rejects 1024)

_cache = {}


def _build(npos):
    import concourse.bass as bass
    import concourse.tile as tile
    from concourse import bacc, mybir

    f32 = mybir.dt.float32
    f16 = mybir.dt.float16
    Exp = mybir.ActivationFunctionType.Exp
    X = mybir.AxisListType.X

    nc = bacc.Bacc("TRN2", target_bir_lowering=False, debug=False)
    yt = nc.dram_tensor("yt", [KAUG, M], f16, kind="ExternalInput").ap()
    xt = nc.dram_tensor("xt", [KAUG, NLOC], f16, kind="ExternalInput").ap()
    o = nc.dram_tensor("o", [128, ITILES], f32, kind="ExternalOutput").ap()

    segs = []
    for gi in range(NGRP):
        g0, g1 = gi * GRP, (gi + 1) * GRP
        if g0 < npos:
            segs.append((g0, min(g1, npos), True))
        if g1 > npos:
            segs.append((max(g0, npos), g1, False))
    npos_segs = sum(1 for s in segs if s[2])
    nseg = len(segs)

    with tile.TileContext(nc) as tc:
        with tc.tile_pool(name="ypool", bufs=1) as ypool, \
             tc.tile_pool(name="xpool", bufs=1) as xpool, \
             tc.tile_pool(name="psum", bufs=2, space="PSUM") as pp, \
             tc.tile_pool(name="parts", bufs=ITILES) as partp, \
             tc.tile_pool(name="small", bufs=2 * ITILES) as smallp, \
             tc.tile_pool(name="res", bufs=1) as resp:

            # y chunks split across two DMA queues (sync + gpsimd) so the
            # first chunk lands early and compute starts while the rest
            # stream in. Real matmuls carry the DMA wait directly (first use
            # of a chunk) plus at most one slot-release wait — within the
            # 2-wait walrus limit.
            YCH = 4096
            yts = []
            for ci in range(M // YCH):
                t = ypool.tile([KAUG, YCH], f16, tag=f"yt{ci}")
                q = nc.sync if ci % 2 == 0 else nc.gpsimd
                q.dma_start(t[:], yt[:, bass.ts(ci, YCH)])
                yts.append(t)
            xts = xpool.tile([KAUG, NLOC], f16, tag="xt")
            nc.sync.dma_start(xts[:], xt[:])

            res = resp.tile([128, ITILES], f32)

            for it in range(ITILES):
                xw = xts[:, bass.ts(it, 128)]
                parts = partp.tile([128, nseg], f32, tag="parts")

                seg_i = 0
                for gi in range(NGRP):
                    ps = pp.tile([128, GRP], f32, tag="ps")
                    g0 = gi * GRP
                    for h in range(GRP // MMN):
                        j0 = g0 + h * MMN
                        ci, off = j0 // YCH, j0 % YCH
                        nc.tensor.matmul(ps[:, bass.ts(h, MMN)], xw,
                                         yts[ci][:, off:off + MMN],
                                         start=True, stop=True)
                    while seg_i < nseg and segs[seg_i][0] < g0 + GRP:
                        s0, s1, _pos = segs[seg_i]
                        seg = ps[:, s0 - g0: s1 - g0]
                        nc.scalar.activation(seg, seg, Exp,
                                             accum_out=parts[:, seg_i:seg_i + 1])
                        seg_i += 1

                possum = smallp.tile([128, 1], f32, tag="pos")
                negsum = smallp.tile([128, 1], f32, tag="neg")
                if npos_segs:
                    nc.vector.reduce_sum(possum[:], parts[:, 0:npos_segs], axis=X)
                else:
                    nc.vector.memset(possum[:], 0.0)
                if nseg - npos_segs:
                    nc.vector.reduce_sum(negsum[:], parts[:, npos_segs:nseg], axis=X)
                else:
                    nc.vector.memset(negsum[:], 0.0)
                nc.vector.tensor_sub(res[:, it:it + 1], possum[:], negsum[:])

            nc.sync.dma_start(o[:], res[:])

    nc.compile()
    return nc


def kernel(x, y_train, alphas, g):
    from concourse.bass_utils import run_bass_kernel_spmd

    x = np.asarray(x, dtype=np.float32)
    y_train = np.asarray(y_train, dtype=np.float32)
    a = np.asarray(alphas, dtype=np.float32).reshape(-1)
    gf = float(np.asarray(g).reshape(-1)[0])

    y2 = np.sum(y_train.astype(np.float64) ** 2, axis=1)
    with np.errstate(divide="ignore"):
        c = -y2 / gf + np.log(np.abs(a.astype(np.float64)))
    c = np.maximum(c, -1e4)

    pos = a >= 0
    order = np.concatenate([np.nonzero(pos)[0], np.nonzero(~pos)[0]])
    npos = int(pos.sum())

    ytab = np.empty((KAUG, M), dtype=np.float64)
    ytab[:DIM] = (2.0 / gf) * y_train[order].T.astype(np.float64)
    co = c[order]
    ch = co.astype(np.float16).astype(np.float64)
    ytab[DIM] = ch
    ytab[DIM + 1] = co - ch
    ytn = ytab.astype(np.float16)

    key = npos
    if key not in _cache:
        _cache[key] = _build(npos)
    nc = _cache[key]

    in_maps = []
    for k in range(NCORES):
        xs = x[k * NLOC:(k + 1) * NLOC]
        xtab = np.empty((KAUG, NLOC), dtype=np.float64)
        xtab[:DIM] = xs.T.astype(np.float64)
        xtab[DIM] = 1.0
        xtab[DIM + 1] = 1.0
        in_maps.append({
            "yt": ytn,
            "xt": xtab.astype(np.float16),
        })

    r = run_bass_kernel_spmd(nc, in_maps, core_ids=list(range(NCORES)))

    x2 = np.sum(x.astype(np.float64) ** 2, axis=1)
    rowscale = np.exp(-x2 / gf)
    out = np.empty(N, dtype=np.float64)
    for k in range(NCORES):
        out[k * NLOC:(k + 1) * NLOC] = r.results[k]["o"].T.reshape(NLOC).astype(np.float64)
    out *= rowscale
    return out.astype(np.float32).reshape(N, 1)


# revision 10
# speedup vs baseline: 2.1270x; 1.1069x over previous
# Gaussian-kernel ridge-regression matvec on 8 Trainium2 cores.
#
#   out_i = sum_j exp(-||x_i - y_j||^2 / g) * alpha_j
#   N=8192 queries, M=16384 train points, DIM=32, g scalar.
#
# Factorization (host prep is O(N+M), device does the O(N*M) part):
#   exp(-(x^2+y^2-2xy)/g)*a_j = exp(-x_i^2/g) * sign(a_j) * exp(s_ij),
#   s_ij = (2/g) x_i.y_j + c_j,   c_j = -y_j^2/g + ln|a_j|
# Train points are host-sorted so sign(a)>0 comes first (npos); the device
# computes s via an augmented K=34 fp16 matmul: rows 0-31 are the (2/g)-scaled
# y dims (x rows are the fp16 query dims), rows 32/33 carry c_j split hi/lo
# in fp16 (x rows 32/33 are 1.0) so c reaches the exp at full accuracy while
# the dot itself is single-pass fp16 (~4e-3 abs err in s, well inside the
# 2e-2 gate). The exp runs on ACT in-place on PSUM with accum_out giving
# per-row sums per pos/neg segment; tiny DVE reduce/sub; one DMA out.
# Row scale exp(-x_i^2/g) is applied on host.
#
# ACT is the bottleneck (1 elem/cycle/lane @1.2GHz): 16K j x 1K i per core
# = 131072 elems/lane ~ 109us + ~293ns/instr pipeline fill. Structure keeps
# ACT saturated: [128,2048] PSUM groups double-buffered (4+4 banks), flat
# (group, itile) iteration so psum-slot reuse never waits a bunched itile
# tail, y streamed in 512-col chunks over 3 DMA queues so compute starts
# ~10us in.

import numpy as np

N, M, DIM, NCORES = 8192, 16384, 32, 8
NLOC = N // NCORES
ITILES = NLOC // 128
GRP = 2048
NGRP = M // GRP
KAUG = DIM + 2
MMN = 512
YCH = 1024

_cache = {}


def _build(npos):
    import concourse.bass as bass
    import concourse.tile as tile
    from concourse import bacc, mybir

    f32 = mybir.dt.float32
    f16 = mybir.dt.float16
    Exp = mybir.ActivationFunctionType.Exp
    X = mybir.AxisListType.X

    nc = bacc.Bacc("TRN2", target_bir_lowering=False, debug=False)
    yt = nc.dram_tensor("yt", [KAUG, M], f16, kind="ExternalInput").ap()
    xt = nc.dram_tensor("xt", [KAUG, NLOC], f16, kind="ExternalInput").ap()
    o = nc.dram_tensor("o", [128, ITILES], f32, kind="ExternalOutput").ap()

    segs = []
    for gi in range(NGRP):
        g0, g1 = gi * GRP, (gi + 1) * GRP
        if g0 < npos:
            segs.append((g0, min(g1, npos), True))
        if g1 > npos:
            segs.append((max(g0, npos), g1, False))
    npos_segs = sum(1 for s in segs if s[2])
    nseg = len(segs)
    # last group index contributing a positive segment
    last_pos_grp = max((s[0] // GRP for s in segs if s[2]), default=-1)

    with tile.TileContext(nc) as tc:
        with tc.tile_pool(name="ypool", bufs=1) as ypool, \
             tc.tile_pool(name="xpool", bufs=1) as xpool, \
             tc.tile_pool(name="psum", bufs=2, space="PSUM") as pp, \
             tc.tile_pool(name="parts", bufs=ITILES) as partp, \
             tc.tile_pool(name="small", bufs=3 * ITILES + 2) as smallp, \
             tc.tile_pool(name="res", bufs=1) as resp:

            # Warm the ACT exp table during the DMA wait (the first real
            # ACTIVATE otherwise eats the ~1.3us ACT_TABLE_LOAD).
            dact = smallp.tile([1, 1], f32, tag="dact")
            nc.vector.memset(dact[:], 0.0)
            nc.scalar.activation(dact[:], dact[:], Exp)

            # x first (needed by every matmul), then y in 512-col chunks
            # round-robined over three DMA queues so chunk 0 lands early and
            # compute starts while the rest stream in. Matmuls carry the DMA
            # wait directly (first use of a chunk) plus at most one
            # slot-release wait — within the 2-wait walrus limit.
            xts = xpool.tile([KAUG, NLOC], f16, tag="xt")
            nc.scalar.dma_start(xts[:], xt[:])
            queues = [nc.sync, nc.gpsimd]
            yts = []
            for ci in range(M // YCH):
                t = ypool.tile([KAUG, YCH], f16, tag=f"yt{ci}", name=f"ytile{ci}")
                queues[ci % 2].dma_start(t[:], yt[:, bass.ts(ci, YCH)])
                yts.append(t)

            res = resp.tile([128, ITILES], f32)
            parts = [partp.tile([128, nseg], f32, tag=f"parts{it}", name=f"parts{it}")
                     for it in range(ITILES)]
            possums = [smallp.tile([128, 1], f32, tag=f"pos{it}", name=f"pos{it}")
                       for it in range(ITILES)]
            negsums = [smallp.tile([128, 1], f32, tag=f"neg{it}", name=f"neg{it}")
                       for it in range(ITILES)]

            seg_base = 0
            for gi in range(NGRP):
                g0 = gi * GRP
                gsegs = [(i, s) for i, s in enumerate(segs)
                         if g0 <= s[0] < g0 + GRP]
                for it in range(ITILES):
                    xw = xts[:, bass.ts(it, 128)]
                    ps = pp.tile([128, GRP], f32, tag="ps")
                    for h in range(GRP // MMN):
                        j0 = g0 + h * MMN
                        ci, off = j0 // YCH, j0 % YCH
                        nc.tensor.matmul(ps[:, bass.ts(h, MMN)], xw,
                                         yts[ci][:, off:off + MMN],
                                         start=True, stop=True)
                    for seg_i, (s0, s1, _pos) in gsegs:
                        seg = ps[:, s0 - g0: s1 - g0]
                        nc.scalar.activation(
                            seg, seg, Exp,
                            accum_out=parts[it][:, seg_i:seg_i + 1])
                    # partial reductions as soon as an itile's pos (or neg)
                    # columns are complete — keeps the tail to one itile.
                    if gi == last_pos_grp and npos_segs:
                        nc.vector.reduce_sum(possums[it][:],
                                             parts[it][:, 0:npos_segs], axis=X)
                    if gi == NGRP - 1:
                        if nseg - npos_segs:
                            nc.vector.reduce_sum(
                                negsums[it][:],
                                parts[it][:, npos_segs:nseg], axis=X)
                        else:
                            nc.vector.memset(negsums[it][:], 0.0)
                        if not npos_segs:
                            nc.vector.memset(possums[it][:], 0.0)
                        nc.vector.tensor_sub(res[:, it:it + 1],
                                             possums[it][:], negsums[it][:])

            nc.sync.dma_start(o[:], res[:])

    nc.compile()
    return nc


def kernel(x, y_train, alphas, g):
    from concourse.bass_utils import run_bass_kernel_spmd

    x = np.asarray(x, dtype=np.float32)
    y_train = np.asarray(y_train, dtype=np.float32)
    a = np.asarray(alphas, dtype=np.float32).reshape(-1)
    gf = float(np.asarray(g).reshape(-1)[0])

    y2 = np.sum(y_train.astype(np.float64) ** 2, axis=1)
    with np.errstate(divide="ignore"):
        c = -y2 / gf + np.log(np.abs(a.astype(np.float64)))
    c = np.maximum(c, -1e4)

    pos = a >= 0
    order = np.concatenate([np.nonzero(pos)[0], np.nonzero(~pos)[0]])
    npos = int(pos.sum())

    ytab = np.empty((KAUG, M), dtype=np.float64)
    ytab[:DIM] = (2.0 / gf) * y_train[order].T.astype(np.float64)
    co = c[order]
    ch = co.astype(np.float16).astype(np.float64)
    ytab[DIM] = ch
    ytab[DIM + 1] = co - ch
    ytn = ytab.astype(np.float16)

    key = npos
    if key not in _cache:
        _cache[key] = _build(npos)
    nc = _cache[key]

    in_maps = []
    for k in range(NCORES):
        xs = x[k * NLOC:(k + 1) * NLOC]
        xtab = np.empty((KAUG, NLOC), dtype=np.float64)
        xtab[:DIM] = xs.T.astype(np.float64)
        xtab[DIM] = 1.0
        xtab[DIM + 1] = 1.0
        in_maps.append({
            "yt": ytn,
            "xt": xtab.astype(np.float16),
        })

    r = run_bass_kernel_spmd(nc, in_maps, core_ids=list(range(NCORES)))

    x2 = np.sum(x.astype(np.float64) ** 2, axis=1)
    rowscale = np.exp(-x2 / gf)
    out = np.empty(N, dtype=np.float64)
    for k in range(NCORES):
        out[k * NLOC:(k + 1) * NLOC] = r.results[k]["o"].T.reshape(NLOC).astype(np.float64)
    out *= rowscale
    return out.astype(np.float32).reshape(N, 1)


# revision 15
# speedup vs baseline: 2.1660x; 1.0184x over previous
# Gaussian-kernel ridge-regression matvec on 8 Trainium2 cores.
#
#   out_i = sum_j exp(-||x_i - y_j||^2 / g) * alpha_j
#   N=8192 queries, M=16384 train points, DIM=32, g scalar.
#
# Factorization (host prep is O(N+M), device does the O(N*M) part):
#   exp(-(x^2+y^2-2xy)/g)*a_j = exp(-x_i^2/g) * sign(a_j) * exp(s_ij),
#   s_ij = (2/g) x_i.y_j + c_j,   c_j = -y_j^2/g + ln|a_j|
# Train points are host-sorted so sign(a)>0 comes first (npos); the device
# computes s via an augmented K=34 fp16 matmul: rows 0-31 are the (2/g)-scaled
# y dims (x rows are the fp16 query dims), rows 32/33 carry c_j split hi/lo
# in fp16 (x rows 32/33 are 1.0) so c reaches the exp at full accuracy while
# the dot itself is single-pass fp16 (~4e-3 abs err in s, well inside the
# 2e-2 gate). The exp runs on ACT in-place on PSUM with accum_out giving
# per-row sums per pos/neg segment; tiny DVE reduce/sub; one DMA out.
# Row scale exp(-x_i^2/g) is applied on host.
#
# ACT is the bottleneck (1 elem/cycle/lane @1.2GHz): 16K j x 1K i per core
# = 131072 elems/lane ~ 109us + ~293ns/instr pipeline fill. Structure keeps
# ACT saturated: [128,2048] PSUM groups double-buffered (4+4 banks), flat
# (group, itile) iteration so psum-slot reuse never waits a bunched itile
# tail, y streamed in 512-col chunks over 3 DMA queues so compute starts
# ~10us in.

import numpy as np

N, M, DIM, NCORES = 8192, 16384, 32, 8
NLOC = N // NCORES
ITILES = NLOC // 128
GRP = 2048
NGRP = M // GRP
KAUG = DIM + 2
MMN = 512
YCH = 1024

_cache = {}


def _build(npos):
    import concourse.bass as bass
    import concourse.tile as tile
    from concourse import bacc, mybir

    f32 = mybir.dt.float32
    f16 = mybir.dt.float16
    Exp = mybir.ActivationFunctionType.Exp
    X = mybir.AxisListType.X

    nc = bacc.Bacc("TRN2", target_bir_lowering=False, debug=False)
    yt = nc.dram_tensor("yt", [KAUG, M], f16, kind="ExternalInput").ap()
    xt = nc.dram_tensor("xt", [KAUG, NLOC], f16, kind="ExternalInput").ap()
    o = nc.dram_tensor("o", [128, ITILES], f32, kind="ExternalOutput").ap()

    segs = []
    for gi in range(NGRP):
        g0, g1 = gi * GRP, (gi + 1) * GRP
        if g0 < npos:
            segs.append((g0, min(g1, npos), True))
        if g1 > npos:
            segs.append((max(g0, npos), g1, False))
    npos_segs = sum(1 for s in segs if s[2])
    nseg = len(segs)
    # last group index contributing a positive segment
    last_pos_grp = max((s[0] // GRP for s in segs if s[2]), default=-1)

    with tile.TileContext(nc) as tc:
        with tc.tile_pool(name="ypool", bufs=1) as ypool, \
             tc.tile_pool(name="xpool", bufs=1) as xpool, \
             tc.tile_pool(name="psum", bufs=2, space="PSUM") as pp, \
             tc.tile_pool(name="parts", bufs=2 * ITILES) as partp, \
             tc.tile_pool(name="small", bufs=3 * ITILES + 2) as smallp, \
             tc.tile_pool(name="res", bufs=1) as resp:

            # Warm the ACT exp table during the DMA wait (the first real
            # ACTIVATE otherwise eats the ~1.3us ACT_TABLE_LOAD).
            dact = smallp.tile([1, 1], f32, tag="dact")
            nc.vector.memset(dact[:], 0.0)
            nc.scalar.activation(dact[:], dact[:], Exp)

            # x first (needed by every matmul), then y in 512-col chunks
            # round-robined over three DMA queues so chunk 0 lands early and
            # compute starts while the rest stream in. Matmuls carry the DMA
            # wait directly (first use of a chunk) plus at most one
            # slot-release wait — within the 2-wait walrus limit.
            xts = xpool.tile([KAUG, NLOC], f16, tag="xt")
            nc.scalar.dma_start(xts[:], xt[:])
            queues = [nc.sync, nc.gpsimd]
            yts = []
            for ci in range(M // YCH):
                t = ypool.tile([KAUG, YCH], f16, tag=f"yt{ci}", name=f"ytile{ci}")
                if ci < 2:
                    # first two chunks split in halves across both queues so
                    # the first matmul group is ready a few us earlier
                    for hh in range(2):
                        queues[hh].dma_start(
                            t[:, bass.ts(hh, YCH // 2)],
                            yt[:, bass.ts(ci * 2 + hh, YCH // 2)])
                else:
                    queues[ci % 2].dma_start(t[:], yt[:, bass.ts(ci, YCH)])
                yts.append(t)

            res = resp.tile([128, ITILES], f32)
            # separate pos/neg partial tiles per itile: the pos reduction
            # (emitted right after the last pos segment) must not create a
            # tile-granularity read hazard against later neg READ_ACCUMULATOR
            # writes, which would stall the ACT queue mid-stream.
            nneg_segs = nseg - npos_segs
            partsp = [partp.tile([128, max(npos_segs, 1)], f32,
                                 tag=f"pp{it}", name=f"pp{it}")
                      for it in range(ITILES)]
            partsn = [partp.tile([128, max(nneg_segs, 1)], f32,
                                 tag=f"pn{it}", name=f"pn{it}")
                      for it in range(ITILES)]
            possums = [smallp.tile([128, 1], f32, tag=f"pos{it}", name=f"pos{it}")
                       for it in range(ITILES)]
            negsums = [smallp.tile([128, 1], f32, tag=f"neg{it}", name=f"neg{it}")
                       for it in range(ITILES)]

            seg_base = 0
            for gi in range(NGRP):
                g0 = gi * GRP
                gsegs = [(i, s) for i, s in enumerate(segs)
                         if g0 <= s[0] < g0 + GRP]
                for it in range(ITILES):
                    xw = xts[:, bass.ts(it, 128)]
                    ps = pp.tile([128, GRP], f32, tag="ps")
                    for h in range(GRP // MMN):
                        j0 = g0 + h * MMN
                        ci, off = j0 // YCH, j0 % YCH
                        nc.tensor.matmul(ps[:, bass.ts(h, MMN)], xw,
                                         yts[ci][:, off:off + MMN],
                                         start=True, stop=True)
                    for seg_i, (s0, s1, pos_) in gsegs:
                        seg = ps[:, s0 - g0: s1 - g0]
                        col = seg_i if pos_ else seg_i - npos_segs
                        tgt = partsp[it] if pos_ else partsn[it]
                        nc.scalar.activation(
                            seg, seg, Exp,
                            accum_out=tgt[:, col:col + 1])
                    # partial reductions as soon as an itile's pos (or neg)
                    # columns are complete — keeps the tail to one itile.
                    if gi == last_pos_grp and npos_segs:
                        nc.vector.reduce_sum(possums[it][:], partsp[it][:],
                                             axis=X)
                    if gi == NGRP - 1:
                        if nneg_segs:
                            nc.vector.reduce_sum(negsums[it][:], partsn[it][:],
                                                 axis=X)
                        else:
                            nc.vector.memset(negsums[it][:], 0.0)
                        if not npos_segs:
                            nc.vector.memset(possums[it][:], 0.0)
                        nc.vector.tensor_sub(res[:, it:it + 1],
                                             possums[it][:], negsums[it][:])

            nc.sync.dma_start(o[:], res[:])

    nc.compile()
    return nc


def kernel(x, y_train, alphas, g):
    from concourse.bass_utils import run_bass_kernel_spmd

    x = np.asarray(x, dtype=np.float32)
    y_train = np.asarray(y_train, dtype=np.float32)
    a = np.asarray(alphas, dtype=np.float32).reshape(-1)
    gf = float(np.asarray(g).reshape(-1)[0])

    y2 = np.sum(y_train.astype(np.float64) ** 2, axis=1)
    with np.errstate(divide="ignore"):
        c = -y2 / gf + np.log(np.abs(a.astype(np.float64)))
    c = np.maximum(c, -1e4)

    pos = a >= 0
    order = np.concatenate([np.nonzero(pos)[0], np.nonzero(~pos)[0]])
    npos = int(pos.sum())

    ytab = np.empty((KAUG, M), dtype=np.float64)
    ytab[:DIM] = (2.0 / gf) * y_train[order].T.astype(np.float64)
    co = c[order]
    ch = co.astype(np.float16).astype(np.float64)
    ytab[DIM] = ch
    ytab[DIM + 1] = co - ch
    ytn = ytab.astype(np.float16)

    key = npos
    if key not in _cache:
        _cache[key] = _build(npos)
    nc = _cache[key]

    in_maps = []
    for k in range(NCORES):
        xs = x[k * NLOC:(k + 1) * NLOC]
        xtab = np.empty((KAUG, NLOC), dtype=np.float64)
        xtab[:DIM] = xs.T.astype(np.float64)
        xtab[DIM] = 1.0
        xtab[DIM + 1] = 1.0
        in_maps.append({
            "yt": ytn,
            "xt": xtab.astype(np.float16),
        })

    r = run_bass_kernel_spmd(nc, in_maps, core_ids=list(range(NCORES)))

    x2 = np.sum(x.astype(np.float64) ** 2, axis=1)
    rowscale = np.exp(-x2 / gf)
    out = np.empty(N, dtype=np.float64)
    for k in range(NCORES):
        out[k * NLOC:(k + 1) * NLOC] = r.results[k]["o"].T.reshape(NLOC).astype(np.float64)
    out *= rowscale
    return out.astype(np.float32).reshape(N, 1)


# revision 18
# speedup vs baseline: 2.2615x; 1.0441x over previous
# Gaussian-kernel ridge-regression matvec on 8 Trainium2 cores.
#
#   out_i = sum_j exp(-||x_i - y_j||^2 / g) * alpha_j
#   N=8192 queries, M=16384 train points, DIM=32, g scalar.
#
# Factorization (host prep is O(N+M), device does the O(N*M) part):
#   exp(-(x^2+y^2-2xy)/g)*a_j = exp(-x_i^2/g) * sign(a_j) * exp(s_ij),
#   s_ij = (2/g) x_i.y_j + c_j,   c_j = -y_j^2/g + ln|a_j|
# Train points are host-sorted so sign(a)>0 comes first (npos); the device
# computes s via an augmented K=34 fp16 matmul: rows 0-31 are the (2/g)-scaled
# y dims (x rows are the fp16 query dims), rows 32/33 carry c_j split hi/lo
# in fp16 (x rows 32/33 are 1.0) so c reaches the exp at full accuracy while
# the dot itself is single-pass fp16 (~4e-3 abs err in s, well inside the
# 2e-2 gate). The exp runs on ACT in-place on PSUM with accum_out giving
# per-row sums per pos/neg segment; tiny DVE reduce/sub; one DMA out.
# Row scale exp(-x_i^2/g) is applied on host.
#
# ACT is the bottleneck (1 elem/cycle/lane @1.2GHz): 16K j x 1K i per core
# = 131072 elems/lane ~ 109us + ~293ns/instr pipeline fill. Structure keeps
# ACT saturated: [128,2048] PSUM groups double-buffered (4+4 banks), flat
# (group, itile) iteration so psum-slot reuse never waits a bunched itile
# tail, y streamed in 512-col chunks over 3 DMA queues so compute starts
# ~10us in.

import numpy as np

N, M, DIM, NCORES = 8192, 16384, 32, 8
NLOC = N // NCORES
ITILES = NLOC // 128
GRP = 2048
NGRP = M // GRP
KAUG = DIM + 2
MMN = 512
YCH = 1024

_cache = {}


def _build(npos):
    import concourse.bass as bass
    import concourse.tile as tile
    from concourse import bacc, mybir

    f32 = mybir.dt.float32
    f16 = mybir.dt.float16
    Exp = mybir.ActivationFunctionType.Exp
    X = mybir.AxisListType.X

    nc = bacc.Bacc("TRN2", target_bir_lowering=False, debug=False)
    yt = nc.dram_tensor("yt", [KAUG, M], f16, kind="ExternalInput").ap()
    xt = nc.dram_tensor("xt", [KAUG, NLOC], f16, kind="ExternalInput").ap()
    o = nc.dram_tensor("o", [128, ITILES], f32, kind="ExternalOutput").ap()

    # One uniform [128, GRP] ACT instr per group (the scheduler mis-orders
    # mixed-size ACT streams, stalling matmuls on deferred slot releases).
    # The group containing the pos/neg boundary accumulates its neg suffix
    # into the "pos" column; a DVE reduce of the exp'd suffix straight from
    # PSUM corrects it:  res = possum - negsum - 2*S.
    bg = npos // GRP          # boundary group (== NGRP means all-pos)
    suf0 = npos % GRP         # suffix start within group bg (0 -> clean split)
    npos_grps = bg + (1 if suf0 else 0)
    nneg_grps = NGRP - npos_grps

    with tile.TileContext(nc) as tc:
        with tc.tile_pool(name="ypool", bufs=1) as ypool, \
             tc.tile_pool(name="xpool", bufs=1) as xpool, \
             tc.tile_pool(name="psum", bufs=2, space="PSUM") as pp, \
             tc.tile_pool(name="parts", bufs=2 * ITILES) as partp, \
             tc.tile_pool(name="small", bufs=3 * ITILES + 2) as smallp, \
             tc.tile_pool(name="res", bufs=1) as resp:

            # Warm the ACT exp table during the DMA wait (the first real
            # ACTIVATE otherwise eats the ~1.3us ACT_TABLE_LOAD).
            dact = smallp.tile([1, 1], f32, tag="dact")
            nc.vector.memset(dact[:], 0.0)
            nc.scalar.activation(dact[:], dact[:], Exp)

            # x first (needed by every matmul), then y in 512-col chunks
            # round-robined over three DMA queues so chunk 0 lands early and
            # compute starts while the rest stream in. Matmuls carry the DMA
            # wait directly (first use of a chunk) plus at most one
            # slot-release wait — within the 2-wait walrus limit.
            xts = xpool.tile([KAUG, NLOC], f16, tag="xt")
            nc.scalar.dma_start(xts[:], xt[:])
            queues = [nc.sync, nc.gpsimd]
            yts = []
            for ci in range(M // YCH):
                t = ypool.tile([KAUG, YCH], f16, tag=f"yt{ci}", name=f"ytile{ci}")
                if ci < 2:
                    # first two chunks split in halves across both queues so
                    # the first matmul group is ready a few us earlier
                    for hh in range(2):
                        queues[hh].dma_start(
                            t[:, bass.ts(hh, YCH // 2)],
                            yt[:, bass.ts(ci * 2 + hh, YCH // 2)])
                else:
                    queues[ci % 2].dma_start(t[:], yt[:, bass.ts(ci, YCH)])
                yts.append(t)

            res = resp.tile([128, ITILES], f32)
            # separate pos/neg partial tiles per itile: the pos reduction
            # (emitted right after the last pos group) must not create a
            # tile-granularity read hazard against later neg READ_ACCUMULATOR
            # writes, which would stall the ACT queue mid-stream.
            partsp = [partp.tile([128, max(npos_grps, 1)], f32,
                                 tag=f"pp{it}", name=f"pp{it}")
                      for it in range(ITILES)]
            partsn = [partp.tile([128, max(nneg_grps, 1)], f32,
                                 tag=f"pn{it}", name=f"pn{it}")
                      for it in range(ITILES)]
            possums = [smallp.tile([128, 1], f32, tag=f"pos{it}", name=f"pos{it}")
                       for it in range(ITILES)]
            negsums = [smallp.tile([128, 1], f32, tag=f"neg{it}", name=f"neg{it}")
                       for it in range(ITILES)]
            sufs = [smallp.tile([128, 1], f32, tag=f"suf{it}", name=f"suf{it}")
                    for it in range(ITILES)]

            for gi in range(NGRP):
                g0 = gi * GRP
                for it in range(ITILES):
                    xw = xts[:, bass.ts(it, 128)]
                    ps = pp.tile([128, GRP], f32, tag="ps")
                    for h in range(GRP // MMN):
                        j0 = g0 + h * MMN
                        ci, off = j0 // YCH, j0 % YCH
                        nc.tensor.matmul(ps[:, bass.ts(h, MMN)], xw,
                                         yts[ci][:, off:off + MMN],
                                         start=True, stop=True)
                    if gi < npos_grps:
                        tgt = partsp[it][:, gi:gi + 1]
                    else:
                        tgt = partsn[it][:, gi - npos_grps:gi - npos_grps + 1]
                    nc.scalar.activation(ps[:], ps[:], Exp, accum_out=tgt)
                    if gi == bg and suf0:
                        # neg suffix of the boundary group, straight from the
                        # exp'd PSUM (cheap DVE op, off the ACT queue)
                        nc.vector.reduce_sum(sufs[it][:], ps[:, suf0:GRP],
                                             axis=X)
                    # partial reductions as soon as an itile's pos (or neg)
                    # columns are complete — keeps the tail to one itile.
                    if gi == npos_grps - 1 and npos_grps:
                        nc.vector.reduce_sum(possums[it][:], partsp[it][:],
                                             axis=X)
                    if gi == NGRP - 1:
                        if nneg_grps:
                            nc.vector.reduce_sum(negsums[it][:], partsn[it][:],
                                                 axis=X)
                        else:
                            nc.vector.memset(negsums[it][:], 0.0)
                        if not npos_grps:
                            nc.vector.memset(possums[it][:], 0.0)
                        nc.vector.tensor_sub(res[:, it:it + 1],
                                             possums[it][:], negsums[it][:])
                        if suf0:
                            nc.vector.tensor_sub(res[:, it:it + 1],
                                                 res[:, it:it + 1],
                                                 sufs[it][:])
                            nc.vector.tensor_sub(res[:, it:it + 1],
                                                 res[:, it:it + 1],
                                                 sufs[it][:])

            nc.sync.dma_start(o[:], res[:])

    nc.compile()
    return nc


def kernel(x, y_train, alphas, g):
    from concourse.bass_utils import run_bass_kernel_spmd

    x = np.asarray(x, dtype=np.float32)
    y_train = np.asarray(y_train, dtype=np.float32)
    a = np.asarray(alphas, dtype=np.float32).reshape(-1)
    gf = float(np.asarray(g).reshape(-1)[0])

    y2 = np.sum(y_train.astype(np.float64) ** 2, axis=1)
    with np.errstate(divide="ignore"):
        c = -y2 / gf + np.log(np.abs(a.astype(np.float64)))
    c = np.maximum(c, -1e4)

    pos = a >= 0
    order = np.concatenate([np.nonzero(pos)[0], np.nonzero(~pos)[0]])
    npos = int(pos.sum())

    ytab = np.empty((KAUG, M), dtype=np.float64)
    ytab[:DIM] = (2.0 / gf) * y_train[order].T.astype(np.float64)
    co = c[order]
    ch = co.astype(np.float16).astype(np.float64)
    ytab[DIM] = ch
    ytab[DIM + 1] = co - ch
    ytn = ytab.astype(np.float16)

    key = npos
    if key not in _cache:
        _cache[key] = _build(npos)
    nc = _cache[key]

    in_maps = []
    for k in range(NCORES):
        xs = x[k * NLOC:(k + 1) * NLOC]
        xtab = np.empty((KAUG, NLOC), dtype=np.float64)
        xtab[:DIM] = xs.T.astype(np.float64)
        xtab[DIM] = 1.0
        xtab[DIM + 1] = 1.0
        in_maps.append({
            "yt": ytn,
            "xt": xtab.astype(np.float16),
        })

    r = run_bass_kernel_spmd(nc, in_maps, core_ids=list(range(NCORES)))

    x2 = np.sum(x.astype(np.float64) ** 2, axis=1)
    rowscale = np.exp(-x2 / gf)
    out = np.empty(N, dtype=np.float64)
    for k in range(NCORES):
        out[k * NLOC:(k + 1) * NLOC] = r.results[k]["o"].T.reshape(NLOC).astype(np.float64)
    out *= rowscale
    return out.astype(np.float32).reshape(N, 1)
